# revision 18
# baseline (speedup 1.0000x reference)
"""Trainium2 Bass kernel for masked attention-pooling (DmasifAttentionModule).

Reference computation (per sample b):
    proj   = x @ W.T + b                  # [N, D]
    scores = proj @ v                     # [N]
    scores = where(mask, scores, -1e9)
    w      = softmax(scores)              # [N]
    out    = w @ x                        # [D]

Optimizations:
  1. scores = x @ (W.T @ v) + (b . v); softmax shift-invariance drops the
     constant, so the projection collapses to a matvec against u = v @ W
     (host-computed, 512 floats).
  2. Host compacts each sample to its valid rows (padded to ncols*128 with
     zero rows); padding rows are killed by a large negative bias folded
     into the score reduction, so their softmax weight is exactly 0.
  3. bf16 x/u stream (halves HBM traffic vs fp32; bf16 pool matmuls run one
     PE pass at 2.4GHz vs fp32's two half-rate passes). Scores/PSUM fp32.
  4. The score reduction s[q] = sum_d x[q,d]u[d] is the expensive part
     (free-axis reduction; the PE can't do it and every DVE/ACT op carries
     a ~300ns issue+drain tax).  HW-measured per-column costs:
       DVE fused STT (x+mb)*u with accum    ~910 ns
       DVE TT pair-product (bf16 2x)        ~460 ns/col
       ACT Identity(p + mb/512) with accum  ~1030 ns
     Columns are split between a fused-STT path on DVE and a TT-product
     + Identity-accum path on ScalarE (ACT_COLS per sample) so both
     engines finish together; each tile's exp is queued one tile late so
     a waiting exp never head-of-line blocks the strict-FIFO ScalarE
     queue (GPSIMD tensor ops are rejected by this backend's codegen).
  5. exp(s - C) per DMA-tile on ScalarE (bf16 out, range-safe); Z partials
     via PE ones-matmul into PSUM (no ACT accum-read overhead); pooling
     via PE matvec accumulation into PSUM [1,512] per sample.
  6. Sample-sequential DMA order: s0 tiles [1,4,4,4,4] columns (small
     first tile primes the pipeline), s1 tiles [4,4,4,4,1] (small last
     tile shortens the end-of-kernel tail). s0's finalize hides under
     s1's stream. Finalize ships raw pool + Z partials in ONE DMA.
  7. Host x layout [s, p, q, d] (partition-major) so every DMA moves
     4KB-contiguous runs per partition.
Host finalize: out = raw_pool / sum(z_partials) per sample (fp32).
"""

import os
import sys

import numpy as np

for _p in ("/opt/trn_rl_repo", "/root/.axon_site/_ro/trn_rl_repo"):
    if os.path.isdir(_p) and _p not in sys.path:
        sys.path.append(_p)

import concourse.bacc as bacc
import concourse.tile as tile
from concourse import mybir
from concourse.bass_utils import run_bass_kernel_spmd

B, N, D = 16, 4096, 512
N_CORES = 8
SPB = B // N_CORES          # samples per core
CPT = 4                     # score columns (of 128 rows) per DMA tile
C_SHIFT = 24.0              # constant exp-range shift (softmax-invariant)
MASKED_INIT = -3.0e8        # masked row score -> exp underflows to exactly 0
ACT_COLS = 9                # per-sample columns reduced on ScalarE (paired)
GPS_COLS = 0                # GPSIMD tensor ops rejected by codegen; keep 0

_F32 = mybir.dt.float32
_BF16 = mybir.dt.bfloat16
_F16 = mybir.dt.float16
_BF16_NP = mybir.dt.np(mybir.dt.bfloat16)
_CACHE = {}


def _tile_lists(ncols, cpt=CPT):
    """Per-sample DMA tile lists. s0: 1-col tile first (fast pipeline
    start); s1: 1-col tile last (short end-of-kernel tail)."""
    if ncols <= 1:
        t = [(0, ncols)]
        return [t, t]
    t0 = [(0, 1)] + [(c0, min(cpt, ncols - c0)) for c0 in range(1, ncols, cpt)]
    t1 = [(c0, min(cpt, ncols - 1 - c0)) for c0 in range(0, ncols - 1, cpt)]
    t1 = t1 + [(ncols - 1, 1)]
    return [t0, t1]


def _chunks_of(tiles):
    """Column chunks (<=2 wide) per tile: [(ti, c_local, w), ...]."""
    out = []
    for ti, (c0, cw) in enumerate(tiles):
        c = 0
        while c < cw:
            w = min(2, cw - c)
            out.append((ti, c, w))
            c += w
    return out


def _spread(n, k):
    """Bresenham: k True flags spread over n slots."""
    return [((i + 1) * k) // n > (i * k) // n for i in range(n)]


def _assign_paths(tiles, act_cols, gps_cols, stt_ok):
    """Per (ti, c_local): 'stt' | 'act'. ACT columns are assigned by chunk
    (pairs share one TT product op)."""
    chunks = _chunks_of(tiles)
    ncols = sum(cw for _, cw in tiles)
    if not stt_ok:
        # degenerate fallback: everything via ACT products (no STT scalar)
        return {(ti, c + j): "act" for ti, c, w in chunks for j in range(w)}
    n_act_chunks = max(0, min(len(chunks), (act_cols + 1) // 2))
    act_flags = _spread(len(chunks), n_act_chunks)
    path = {}
    for (ti, c, w), on_act in zip(chunks, act_flags):
        for j in range(w):
            path[(ti, c + j)] = "act" if on_act else "stt"
    return path


def _build_program(ncols, loop_n=None, act_cols=ACT_COLS, gps_cols=GPS_COLS,
                   stt_ok=True):
    ncp = ncols + (ncols & 1)   # even-padded col stride for s/e tiles
    tlists = _tile_lists(ncols)

    nc = bacc.Bacc("TRN2", target_bir_lowering=False, debug=False)
    x = nc.dram_tensor("x", [SPB, 128, ncols, D], _BF16,
                       kind="ExternalInput").ap()
    mbs = nc.dram_tensor("mbs", [SPB, 128, ncols], _F32,
                         kind="ExternalInput").ap()
    mba = nc.dram_tensor("mba", [SPB, 128, ncols], _F32,
                         kind="ExternalInput").ap()
    u = nc.dram_tensor("u", [128, 2, D], _BF16, kind="ExternalInput").ap()
    outz = nc.dram_tensor("outz", [SPB, D + ncols], _F32,
                          kind="ExternalOutput").ap()

    with tile.TileContext(nc) as tc:
        with (
            tc.tile_pool(name="xp", bufs=1) as xp,
            tc.tile_pool(name="singles", bufs=1) as sg,
            tc.tile_pool(name="prod", bufs=4) as pp,
            tc.tile_pool(name="smalls", bufs=2) as sm,
            tc.tile_pool(name="ps", bufs=1, space="PSUM") as psp,
        ):
            ones32 = sg.tile([128, 1], _F32)
            nc.vector.memset(ones32[:], 1.0)
            ones16 = sg.tile([128, 1], _BF16)
            nc.vector.memset(ones16[:], 1.0)
            shift_sb = sg.tile([128, 1], _F32)
            nc.vector.memset(shift_sb[:], -C_SHIFT)
            warm = sg.tile([128, 1], _F32)
            # Pull the exp table-set load to t=0, under the init DMAs.
            nc.scalar.activation(warm[:], ones32[:],
                                 mybir.ActivationFunctionType.Exp)

            u_sb = sg.tile([128, 2, D], _BF16)
            nc.sync.dma_start(out=u_sb[:], in_=u[:])
            mbs_sb = sg.tile([128, SPB, ncols], _F32)
            nc.sync.dma_start(out=mbs_sb[:],
                              in_=mbs.rearrange("s p c -> p s c"))
            mba_sb = sg.tile([128, SPB, ncols], _F32)
            nc.sync.dma_start(out=mba_sb[:],
                              in_=mba.rearrange("s p c -> p s c"))

            s_sb = sg.tile([128, SPB, ncp], _F32)
            e_sb = sg.tile([128, SPB, ncp], _BF16)
            junk_dve = sg.tile([128, D], _BF16)
            junk_act = sg.tile([128, D], _BF16)
            junk_gps = sg.tile([128, D], _BF16)

            pool_ps = {}
            z_ps = {}
            for s in range(SPB):
                pool_ps[s] = psp.tile([1, D], _F32, name=f"pool_ps_{s}")
                z_ps[s] = psp.tile([1, ncp], _F32, name=f"z_ps_{s}")

            paths = [_assign_paths(tlists[s], act_cols, gps_cols, stt_ok)
                     for s in range(SPB)]
            ctx = (nc, xp, pp, sm, x, outz, u_sb, mbs_sb, mba_sb, ones16,
                   shift_sb, s_sb, e_sb, junk_dve, junk_act, junk_gps,
                   pool_ps, z_ps, tlists, paths, ncols)

            if loop_n is not None:
                with tc.For_i(0, loop_n, 1) as _i:
                    _emit_iteration(*ctx)
            else:
                _emit_iteration(*ctx)

    nc.compile()
    return nc


def _emit_iteration(nc, xp, pp, sm, x, outz, u_sb, mbs_sb, mba_sb, ones16,
                    shift_sb, s_sb, e_sb, junk_dve, junk_act, junk_gps,
                    pool_ps, z_ps, tlists, paths, ncols):
    Exp = mybir.ActivationFunctionType.Exp
    Copy = mybir.ActivationFunctionType.Copy
    Ident = mybir.ActivationFunctionType.Identity
    add = mybir.AluOpType.add
    mult = mybir.AluOpType.mult

    # Sample-sequential DMA order; compute chases tile by tile.
    order = [(s, ti) for s in range(SPB) for ti in range(len(tlists[s]))]
    x_tiles = {}
    for s, ti in order:
        c0, cw = tlists[s][ti]
        t = xp.tile([128, cw, D], _BF16, name=f"xt_{s}_{ti}", bufs=1)
        nc.sync.dma_start(out=t[:], in_=x[s, :, c0:c0 + cw, :])
        x_tiles[(s, ti)] = t


    def _finalize(s):
        # Ship raw pool + Z partials in one DMA; host does out = raw/Z.
        oz = sm.tile([1, D + ncols], _F32, name=f"oz_{s}")
        nc.scalar.activation(oz[0:1, 0:D], pool_ps[s][:], Copy)
        nc.scalar.activation(oz[0:1, D:D + ncols], z_ps[s][0:1, 0:ncols],
                             Copy)
        nc.sync.dma_start(out=outz[s:s + 1, :], in_=oz[:])

    # exp/Z/pool run per GROUP of up to 2 tiles (fewer ScalarE ops); a
    # group is emitted only after the NEXT tile's reductions are queued so
    # a waiting exp never head-of-line blocks the strict-FIFO ScalarE
    # queue while its DVE-side inputs finish. 1-col tiles group alone so
    # the end-of-kernel tail stays short.
    groups = []
    for gs in range(SPB):
        nt = len(tlists[gs])
        g = []
        for gti in range(nt):
            g.append(gti)
            cw = tlists[gs][gti][1]
            nxt1 = gti + 1 < nt and tlists[gs][gti + 1][1] == 1
            if len(g) == 2 or gti == nt - 1 or cw == 1 or nxt1:
                groups.append((gs, g))
                g = []
    by_last = {(gs, g[-1]): (gs, g) for gs, g in groups}

    def _softmax_pool(s, g):
        """exp + Z + pooling matmuls for one tile group."""
        tiles = tlists[s]
        g0 = tiles[g[0]][0]
        gw = sum(tiles[t][1] for t in g)
        # e = exp(s - C); padding rows arrive ~MASKED -> exp == 0
        nc.scalar.activation(e_sb[:, s, g0:g0 + gw], s_sb[:, s, g0:g0 + gw],
                             Exp, bias=shift_sb[:])
        # Z partials on PE: ones^T @ e_group -> z_ps[1, gw]
        nc.tensor.matmul(z_ps[s][0:1, g0:g0 + gw], ones16[:],
                         e_sb[:, s, g0:g0 + gw], start=True, stop=True)
        # pooling: accumulate e_col^T @ x_chunk into PSUM [1, D]
        for ti in g:
            xt = x_tiles[(s, ti)]
            c0, cw = tiles[ti]
            for c in range(cw):
                nc.tensor.matmul(
                    pool_ps[s][:],
                    e_sb[:, s, c0 + c:c0 + c + 1],
                    xt[:, c, :],
                    start=(ti == 0 and c == 0),
                    stop=(ti == len(tiles) - 1 and c == cw - 1),
                )
        if g[-1] == len(tiles) - 1:
            _finalize(s)

    for i, (s, ti) in enumerate(order):
        xt = x_tiles[(s, ti)]
        tiles = tlists[s]
        path = paths[s]
        c0, cw = tiles[ti]
        # pair-products (DVE TT, bf16 2x) for ACT-path columns
        p_t = None
        c = 0
        while c < cw:
            w = 2 if (c + 1 < cw and path[(ti, c)] == "act"
                      and path[(ti, c + 1)] == "act") else 1
            if path[(ti, c)] == "act":
                if p_t is None:
                    p_t = pp.tile([128, cw, D], _F16, name=f"p_{s}_{ti}")
                nc.vector.tensor_tensor(
                    p_t[:, c:c + w, :], xt[:, c:c + w, :],
                    u_sb[:, 0:w, :], op=mult)
            c += w
        for c in range(cw):
            col = c0 + c
            pth = path[(ti, c)]
            if pth == "act":
                nc.scalar.activation(junk_act[:], p_t[:, c, :], Ident,
                                     bias=mba_sb[:, s, col:col + 1],
                                     accum_out=s_sb[:, s, col:col + 1])
            else:
                nc.vector.scalar_tensor_tensor(
                    out=junk_dve[:], in0=xt[:, c, :],
                    scalar=mbs_sb[:, s, col:col + 1],
                    in1=u_sb[:, 0, :], op0=add, op1=mult,
                    accum_out=s_sb[:, s, col:col + 1])
        # emit the group whose last tile was the PREVIOUS order entry
        if i > 0 and order[i - 1] in by_last:
            _softmax_pool(*by_last[order[i - 1]])
    _softmax_pool(*by_last[order[-1]])


def _get_program(key):
    if key not in _CACHE:
        ncols, stt_ok = key
        _CACHE[key] = _build_program(ncols, stt_ok=stt_ok)
    return _CACHE[key]


def _prep_inputs(x, flat_mask, W, v):
    """Compact to valid rows, bf16-cast, partition-major layout."""
    x = np.ascontiguousarray(x, dtype=np.float32)
    flat_mask = np.asarray(flat_mask)
    W = np.asarray(W, dtype=np.float32)
    v = np.asarray(v, dtype=np.float32)
    # scores = x @ u + (b . v); the constant drops by softmax invariance.
    u32 = (v @ W).astype(np.float32)
    u = u32.astype(_BF16_NP)
    u2 = np.ascontiguousarray(np.broadcast_to(u, (128, 2, D)))
    s_u = float(u.astype(np.float64).sum())   # sum of the bf16 u the HW sees
    stt_ok = abs(s_u) > 1e-3

    idxs = [np.nonzero(flat_mask[b] == 1)[0] for b in range(B)]
    counts = np.array([len(ix) for ix in idxs])
    ncols = max(1, int(-(-counts.max() // 128)))
    ncap = ncols * 128

    masked_stt = np.float32(MASKED_INIT / s_u) if stt_ok else np.float32(0)
    xc = np.zeros((B, ncap, D), dtype=_BF16_NP)
    mbs = np.full((B, ncap), masked_stt, dtype=np.float32)
    mba = np.full((B, ncap), np.float32(MASKED_INIT / D), dtype=np.float32)
    for b in range(B):
        cnt = counts[b]
        if cnt:
            xc[b, :cnt] = x[b, idxs[b]]
            mbs[b, :cnt] = 0.0
            mba[b, :cnt] = 0.0
    # [B, ncap] -> [B, 128, ncols]: row = q*128 + p -> [b, p, q]
    xc = np.ascontiguousarray(
        xc.reshape(B, ncols, 128, D).transpose(0, 2, 1, 3))
    mbs = np.ascontiguousarray(mbs.reshape(B, ncols, 128).transpose(0, 2, 1))
    mba = np.ascontiguousarray(mba.reshape(B, ncols, 128).transpose(0, 2, 1))

    in_maps = []
    for core in range(N_CORES):
        lo = core * SPB
        in_maps.append({
            "x": np.ascontiguousarray(xc[lo:lo + SPB]),
            "mbs": np.ascontiguousarray(mbs[lo:lo + SPB]),
            "mba": np.ascontiguousarray(mba[lo:lo + SPB]),
            "u": u2,
        })
    meta = {"ncols": ncols, "counts": counts, "stt_ok": stt_ok}
    return in_maps, meta


def kernel(x, flat_mask, W, b, v, **_unused):
    in_maps, meta = _prep_inputs(x, flat_mask, W, v)
    nc = _get_program((meta["ncols"], meta["stt_ok"]))
    res = run_bass_kernel_spmd(nc, in_maps, core_ids=list(range(N_CORES)))
    outz = np.concatenate([res.results[i]["outz"] for i in range(N_CORES)],
                          axis=0)
    raw = outz[:, :D]
    z = outz[:, D:].sum(axis=1, dtype=np.float32)
    with np.errstate(divide="ignore", invalid="ignore"):
        out = (raw / z[:, None]).astype(np.float32)
    if (meta["counts"] == 0).any():
        # Reference semantics for an all-masked sample: uniform mean pool.
        x = np.asarray(x, dtype=np.float32)
        for bi in np.nonzero(meta["counts"] == 0)[0]:
            out[bi] = x[bi].mean(axis=0)
    return out
